# revision 29
# baseline (speedup 1.0000x reference)
"""Trainium2 distributed kernel for a dense transformer block (8 NeuronCores).

Sharding: tokens are data-parallel for LN/QKV/proj/MLP (512 tokens/core,
causal-balanced pairing: core i owns batch0 chunk i and batch1 chunk 7-i),
attention is head-parallel (2 heads/core) via AllToAll exchanges of Q/K/V.

v2 structure (vs v1):
  - Both AllToAlls are split into per-batch halves so they overlap compute
    (QKV for batch1 hides A2A#1a; attention on batch1 hides A2A#2a).
  - Softmax normalization is deferred: unnormalized O plus the denominator
    row ride A2A#2; the reciprocal+broadcast happens once per core after
    the exchange, off the attention critical path.
  - LayerNorm gamma/beta are folded into the weights host-side; rstd uses
    reciprocal_approx_fast; 1/D is folded into the stats ones-vector.
  - All copies/bias-adds are explicit nc.vector ops (scalar engine is
    reserved for Exp and Gelu).
"""

import sys

sys.path.insert(0, "/opt/trn_rl_repo")

import numpy as np
import ml_dtypes

NCORES = 8
D = 1024
H = 16
DH = 64
HL = H // NCORES  # heads per core = 2
B = 2
S = 2048
T = 512  # tokens per core
CH = 256  # token chunk (half of T = one batch's chunk)
DFF = 4096
P = 128
QR, KR, VR = 128, 128, 130  # slot row counts: qT, kT, packed-v regions
SLOT = QR + KR + VR  # 386
SLOT2 = 130  # a2 slot: 2 heads x (64 dims + 1 denom row)
EPS = 1e-5

_CACHE = {}
TRACE = False


def _emit(nc, tc, env):
    from contextlib import ExitStack

    from concourse import bass, mybir

    f32 = mybir.dt.float32
    bf16 = mybir.dt.bfloat16
    Alu = mybir.AluOpType
    AFT = mybir.ActivationFunctionType

    (xT, wT, wpT, wuT, wdT, out) = env["params"]
    (a1i, a1o, a2i, a2o) = env["bounce"]  # dicts {0: tensorA, 1: tensorB}
    c = env["consts"]
    pools = env["pools"]
    rg = [list(range(NCORES))]
    vec = pools["vec"]

    # ---- persistent psum pool (bank budget: mm 3; att pools are scoped) ----
    top = env["top"]
    mm_ps = top.enter_context(tc.tile_pool(name="mm_ps", bufs=3, space="PSUM"))

    def layer_norm_T(x_tiles, xb_tiles, out_pool, pfx):
        """x_tiles: 8x[128,T] (f32 or None), xb_tiles: 8x[128,T] bf16 casts
        (made here if None). Returns 8 normalized bf16 z tiles [128,T]."""
        with tc.tile_pool(name=f"lnst{pfx}", bufs=1, space="PSUM") as lnp, tc.tile_pool(
            name=f"lntmp{pfx}", bufs=3
        ) as tmp_p:
            if xb_tiles is None:
                xb_tiles = []
                for dk in range(8):
                    xb = pools[f"xb{pfx}"].tile([P, T], bf16, name="xb", tag="xb")
                    nc.scalar.activation(xb[:], x_tiles[dk][:], AFT.Copy)
                    xb_tiles.append(xb)
            ps_sum = lnp.tile([1, T], f32, name="ps_sum", tag="ps_sum")
            ps_sq = lnp.tile([1, T], f32, name="ps_sq", tag="ps_sq")
            for dk in range(8):
                nc.tensor.matmul(
                    ps_sum[:], c["ones_d"][:], xb_tiles[dk][:],
                    start=(dk == 0), stop=(dk == 7),
                )
                sq = tmp_p.tile([P, T], bf16, name="sq", tag="sq")
                nc.vector.tensor_tensor(sq[:], xb_tiles[dk][:], xb_tiles[dk][:], Alu.mult)
                nc.tensor.matmul(
                    ps_sq[:], c["ones_d"][:], sq[:], start=(dk == 0), stop=(dk == 7)
                )
            # mu = ps_sum (1/D folded in), msq = ps_sq
            mu_s = vec.tile([1, T], f32, name="mu_s", tag="lnvec")
            nc.vector.tensor_copy(mu_s[:], ps_sum[:])
            mu2 = vec.tile([1, T], f32, name="mu2", tag="lnvec")
            nc.vector.tensor_tensor(mu2[:], mu_s[:], mu_s[:], Alu.mult)
            var = vec.tile([1, T], f32, name="var", tag="lnvec")
            nc.vector.tensor_tensor(var[:], ps_sq[:], mu2[:], Alu.subtract)
            nc.vector.tensor_scalar(var[:], var[:], EPS, None, Alu.add)
            rvar = vec.tile([1, T], f32, name="rvar", tag="lnvec")
            nc.vector.reciprocal(rvar[:], var[:])
            rstd_c = vec.tile([1, T], bf16, name="rstd_c", tag="lnvec")
            nc.scalar.activation(rstd_c[:], rvar[:], AFT.Sqrt)
            mur_c = vec.tile([1, T], bf16, name="mur_c", tag="lnvec")
            with nc.allow_low_precision(reason="ln mean*rstd bcast"):
                nc.vector.tensor_tensor(mur_c[:], mu_s[:], rstd_c[:], Alu.mult)
            rstd_b = lnp.tile([P, T], f32, name="rstd_b", tag="rstd_b")
            nc.tensor.matmul(rstd_b[:], c["ones_row"][:], rstd_c[:], start=True, stop=True)
            mur_b = lnp.tile([P, T], f32, name="mur_b", tag="mur_b")
            nc.tensor.matmul(mur_b[:], c["ones_row"][:], mur_c[:], start=True, stop=True)
            # bf16 SBUF copies of the broadcasts so applies hit DVE 2-byte modes
            rstd_bs = tmp_p.tile([P, T], bf16, name="rstd_bs", tag="rstd_bs")
            nc.vector.tensor_copy(rstd_bs[:], rstd_b[:])
            mur_bs = tmp_p.tile([P, T], bf16, name="mur_bs", tag="mur_bs")
            nc.vector.tensor_copy(mur_bs[:], mur_b[:])
            outs = []
            for dk in range(8):
                t1 = tmp_p.tile([P, T], bf16, name="lnt1", tag="lnt1")
                nc.vector.tensor_tensor(t1[:], xb_tiles[dk][:], rstd_bs[:], Alu.mult)
                o = out_pool.tile([P, T], bf16, name="ln_out", tag="ln_out")
                nc.vector.tensor_tensor(o[:], t1[:], mur_bs[:], Alu.subtract)
                outs.append(o)
            return outs

    # ================= load x, LN1 =================
    x_tiles = []
    for dk in range(8):
        xt = pools["xt"].tile([P, T], f32, name="xt", tag="xt")
        nc.sync.dma_start(xt[:], xT[dk * P : (dk + 1) * P, :])
        x_tiles.append(xt)
    h_tiles = layer_norm_T(x_tiles, None, pools["ht"], "a")

    # ================= QKV per half + A2A#1 =================
    # half hb: token cols hb*CH..(hb+1)*CH of this core's T tokens
    att_scope = ExitStack()
    att_s = att_scope.enter_context(tc.tile_pool(name="att_s", bufs=3, space="PSUM"))
    with tc.tile_pool(name="wqk", bufs=32) as wqk_p, tc.tile_pool(
        name="stg", bufs=8
    ) as stg_p, tc.tile_pool(name="vst", bufs=6) as vst_p:
        wts_qk = []  # prefetch all qk weights once (reused for both halves)
        for blk in range(4):
            row = []
            for dk in range(8):
                wt = wqk_p.tile([P, 512], bf16, name="wqk", tag="wqk")
                nc.sync.dma_start(
                    wt[:], wT[dk * P : (dk + 1) * P, blk * 512 : (blk + 1) * 512]
                )
                row.append(wt)
            wts_qk.append(row)
        wts_v = []
        for jc in range(2):
            row = []
            for dk in range(8):
                wt = wqk_p.tile([P, 512], bf16, name="wv", tag="wv", bufs=16)
                nc.sync.dma_start(
                    wt[:],
                    wT[dk * P : (dk + 1) * P, 2048 + jc * 512 : 2048 + (jc + 1) * 512],
                )
                row.append(wt)
            wts_v.append(row)

        for hb in range(2):
            a1x = a1i[hb]
            cols = slice(hb * CH, (hb + 1) * CH)
            # q/k: 16 output blocks of 128 dims
            for blk in range(4):
                for jl in range(4):
                    jt = blk * 4 + jl  # 0..15 (0-7 q, 8-15 k)
                    ps = mm_ps.tile([P, CH], f32, name="qk_ps", tag="mm")
                    for dk in range(8):
                        nc.tensor.matmul(
                            ps[:],
                            wts_qk[blk][dk][:, jl * P : (jl + 1) * P],
                            h_tiles[dk][:, cols],
                            start=(dk == 0), stop=(dk == 7),
                        )
                    stg = stg_p.tile([P, CH], bf16, name="stg", tag="stg")
                    nc.vector.tensor_scalar(
                        stg[:], ps[:], c["bqk"][:, jt : jt + 1], None, Alu.add
                    )
                    r0 = jt * SLOT if jt < 8 else (jt - 8) * SLOT + QR
                    nc.sync.dma_start(a1x[r0 : r0 + P, :], stg[:])
            # v: out [128 tok, 512 vdim], token chunks tt within this half
            for jc in range(2):
                for tt in range(2):
                    ps = mm_ps.tile([P, 512], f32, name="v_ps", tag="mm")
                    t0 = hb * CH + tt * P
                    for dk in range(8):
                        nc.tensor.matmul(
                            ps[:], h_tiles[dk][:, t0 : t0 + P], wts_v[jc][dk][:],
                            start=(dk == 0), stop=(dk == 7),
                        )
                    for sl in range(4):
                        slot = jc * 4 + sl
                        vt = vst_p.tile([P, VR], bf16, name="vst", tag="vst")
                        for lh in range(HL):
                            cc = slot * P + lh * DH - jc * 512
                            nc.vector.tensor_tensor(
                                vt[:, lh * 65 : lh * 65 + DH],
                                ps[:, cc : cc + DH],
                                c["bv"][:, slot * P + lh * DH : slot * P + lh * DH + DH],
                                Alu.add,
                            )
                            nc.vector.memset(vt[:, lh * 65 + DH : lh * 65 + DH + 1], 1.0)
                        off = (slot * SLOT + QR + KR) * CH + (tt * P) * VR
                        dst = bass.AP(a1x, off, [[VR, P], [1, 65 * HL]])
                        nc.sync.dma_start(dst, vt[:])
            nc.gpsimd.collective_compute(
                "AllToAll", mybir.AluOpType.bypass, replica_groups=rg,
                ins=[a1x.ap().opt()], outs=[a1o[hb].ap().opt()],
            )

    # ================= attention per batch + A2A#2 =================
    load_anchor = {}  # b -> last attention-input load DMA (gates weight prefetch)
    with tc.tile_pool(name="att_o", bufs=2, space="PSUM") as att_o, tc.tile_pool(
        name="kv", bufs=16
    ) as kv_p, tc.tile_pool(name="qe", bufs=8) as qe_p, tc.tile_pool(
        name="a2stg", bufs=16
    ) as a2s_p:
        for b in range(B):
            a1x = a1o[b]
            a2x = a2i[b]
            stage = [
                [
                    a2s_p.tile([65, CH], bf16, name=f"a2stg{b}_{j}_{lh}", tag="a2stg", bufs=32)
                    for lh in range(HL)
                ]
                for j in range(8)
            ]
            for lh in range(HL):
                k_ts, v_ts = [], []
                for kc in range(8):
                    slot = kc if b == 0 else 7 - kc
                    kt = kv_p.tile([DH, CH], bf16, name="kt", tag="kt")
                    nc.sync.dma_start(
                        kt[:],
                        a1x[slot * SLOT + QR + lh * DH : slot * SLOT + QR + (lh + 1) * DH, :],
                    )
                    k_ts.append(kt)
                    for sub in range(2):
                        vt = kv_p.tile([P, 65], bf16, name="vt", tag="vt", bufs=32)
                        off = (slot * SLOT + QR + KR) * CH + (sub * P) * VR + lh * 65
                        vsrc = bass.AP(a1x, off, [[VR, P], [1, 65]])
                        nc.sync.dma_start(vt[:], vsrc)
                        v_ts.append(vt)
                for pr in range(4):  # query-chunk pairs (2pr, 2pr+1)
                    q0, q1 = 2 * pr, 2 * pr + 1
                    s0 = q0 if b == 0 else 7 - q0
                    s1 = q1 if b == 0 else 7 - q1
                    qt = qe_p.tile([DH, 2 * CH], bf16, name="qt", tag="qt")
                    nc.sync.dma_start(
                        qt[:, 0:CH],
                        a1x[s0 * SLOT + lh * DH : s0 * SLOT + (lh + 1) * DH, :],
                    )
                    load_anchor[b] = nc.sync.dma_start(
                        qt[:, CH : 2 * CH],
                        a1x[s1 * SLOT + lh * DH : s1 * SLOT + (lh + 1) * DH, :],
                    )
                    po = att_o.tile([65, 2 * CH], f32, name="o_ps", tag="o")
                    n_mm = 2 * (q1 + 1)
                    mi = 0
                    for kc in range(q1 + 1):
                        for sub in range(2):
                            ps = att_s.tile([P, 2 * CH], f32, name="s_ps", tag="s")
                            nc.tensor.matmul(
                                ps[:], k_ts[kc][:, sub * P : (sub + 1) * P], qt[:],
                                start=True, stop=True,
                            )
                            E = qe_p.tile([P, 2 * CH], bf16, name="E", tag="E")
                            if kc == q1:
                                # future keys for q0-half: zero; diagonal for q1-half
                                nc.scalar.activation(
                                    E[:, CH : 2 * CH], ps[:, CH : 2 * CH],
                                    AFT.Exp, scale=0.125,
                                )
                                nc.vector.memset(E[:, 0:CH], 0.0)
                                nc.vector.tensor_tensor(
                                    E[:, CH : 2 * CH], E[:, CH : 2 * CH],
                                    c["tri"][sub][:], Alu.mult,
                                )
                            else:
                                nc.scalar.activation(E[:], ps[:], AFT.Exp, scale=0.125)
                                if kc == q0:  # diagonal for q0-half only
                                    nc.vector.tensor_tensor(
                                        E[:, 0:CH], E[:, 0:CH], c["tri"][sub][:], Alu.mult
                                    )
                            nc.tensor.matmul(
                                po[:], v_ts[kc * 2 + sub][:], E[:],
                                start=(mi == 0), stop=(mi == n_mm - 1),
                            )
                            mi += 1
                    # stage unnormalized O + denominator row (row 64)
                    for half, sq in ((0, s0), (1, s1)):
                        nc.vector.tensor_copy(
                            stage[sq][lh][:],
                            po[:, half * CH : (half + 1) * CH],
                        )
            for j in range(8):
                for lh in range(HL):
                    nc.sync.dma_start(
                        a2x[j * SLOT2 + lh * 65 : j * SLOT2 + (lh + 1) * 65, :],
                        stage[j][lh][:],
                    )
            nc.gpsimd.collective_compute(
                "AllToAll", mybir.AluOpType.bypass, replica_groups=rg,
                ins=[a2x.ap().opt()], outs=[a2o[b].ap().opt()],
            )

    # ================= post-norm + proj + residual1 =================
    from concourse.bass import _add_dep_helper

    def gate(dma_inst, anchor):
        """Order a weight-prefetch DMA after the attention input loads so it
        doesn't hog the DMA queues during the collective window."""
        _add_dep_helper(
            dma_inst.ins, anchor.ins, sync=False, reason="defer weight prefetch"
        )

    x1_tiles = []
    with tc.tile_pool(name="wp", bufs=16) as wp_p, tc.tile_pool(
        name="onrm", bufs=6
    ) as onrm_p, tc.tile_pool(name="obf", bufs=8) as obf_p:
        wpt = {}
        for dk in range(8):
            for db in range(2):
                wt = wp_p.tile([P, 512], bf16, name="wp", tag="wp")
                gate(
                    nc.sync.dma_start(
                        wt[:], wpT[dk * P : (dk + 1) * P, db * 512 : (db + 1) * 512]
                    ),
                    load_anchor[0],
                )
                wpt[(dk, db)] = wt
        o_bf = []
        for dk in range(8):
            t = obf_p.tile([P, T], bf16, name="obf", tag="obf")
            o_bf.append(t)
        for hb in range(2):
            a2x = a2o[hb]
            # denominators: [16, CH] (core c' rows lh*65+64)
            den_bf = onrm_p.tile([16, CH], bf16, name="den_bf", tag="den_bf")
            dsrc = bass.AP(a2x, DH * CH, [[SLOT2 * CH, 8], [65 * CH, 2], [1, CH]])
            nc.sync.dma_start(den_bf[:], dsrc)
            den = onrm_p.tile([16, CH], f32, name="den", tag="den")
            nc.vector.tensor_copy(den[:], den_bf[:])
            rec = onrm_p.tile([16, CH], f32, name="rec", tag="den")
            nc.vector.reciprocal(rec[:], den[:])
            rec_c = onrm_p.tile([16, CH], bf16, name="rec_c", tag="den_bf")
            with nc.allow_low_precision(reason="softmax denom bcast"):
                nc.vector.tensor_copy(rec_c[:], rec[:])
            for dk in range(8):
                o_un = onrm_p.tile([P, CH], bf16, name="o_un", tag="o_un")
                osrc = bass.AP(a2x, dk * SLOT2 * CH, [[65 * CH, 2], [CH, DH], [1, CH]])
                nc.sync.dma_start(o_un[:], osrc)
                rec_b = att_s.tile([P, CH], f32, name="rec_b", tag="s")
                nc.tensor.matmul(
                    rec_b[:], c["sel"][:, dk * P : (dk + 1) * P], rec_c[:],
                    start=True, stop=True,
                )
                nc.vector.tensor_tensor(
                    o_bf[dk][:, hb * CH : (hb + 1) * CH], o_un[:], rec_b[:], Alu.mult
                )
        x1_tiles = [
            pools["x1"].tile([P, T], f32, name="x1", tag="x1") for _ in range(8)
        ]
        for hb in range(2):  # proj per half so half A overlaps A2A#2B
            cols = slice(hb * CH, (hb + 1) * CH)
            for do in range(8):
                ps = mm_ps.tile([P, CH], f32, name="p_ps", tag="mm")
                for dk in range(8):
                    nc.tensor.matmul(
                        ps[:],
                        wpt[(dk, do // 4)][:, (do % 4) * P : (do % 4 + 1) * P],
                        o_bf[dk][:, cols],
                        start=(dk == 0), stop=(dk == 7),
                    )
                x1 = x1_tiles[do]
                nc.vector.tensor_scalar(
                    x1[:, cols], ps[:], c["bp"][:, do : do + 1], None, Alu.add
                )
                nc.vector.tensor_tensor(
                    x1[:, cols], x1[:, cols], x_tiles[do][:, cols], Alu.add
                )
    att_scope.close()  # frees att_s psum banks before LN2 opens its pool

    # ================= LN2 =================
    h2_tiles = layer_norm_T(x1_tiles, None, pools["h2"], "b")

    # ================= MLP up + gelu =================
    gu_tiles = []
    with tc.tile_pool(name="wu", bufs=16) as wu_p:
        for jb in range(8):
            wut = []
            for dk in range(8):
                wt = wu_p.tile([P, 512], bf16, name="wu", tag="wu")
                gate(
                    nc.sync.dma_start(
                        wt[:], wuT[dk * P : (dk + 1) * P, jb * 512 : (jb + 1) * 512]
                    ),
                    load_anchor[1],
                )
                wut.append(wt)
            for jl in range(4):
                j = jb * 4 + jl
                ps = mm_ps.tile([P, T], f32, name="u_ps", tag="mm")
                for dk in range(8):
                    nc.tensor.matmul(
                        ps[:], wut[dk][:, jl * P : (jl + 1) * P], h2_tiles[dk][:],
                        start=(dk == 0), stop=(dk == 7),
                    )
                gu = pools["gu"].tile([P, T], bf16, name="gu", tag="gu")
                nc.scalar.activation(
                    gu[:], ps[:], AFT.Gelu_apprx_tanh, bias=c["bu"][:, j : j + 1]
                )
                gu_tiles.append(gu)

    # ================= MLP down + residual2 =================
    with tc.tile_pool(name="wd", bufs=34) as wd_p:
        for db in range(2):
            wdt = []
            for j in range(32):
                wt = wd_p.tile([P, 512], bf16, name="wd", tag="wd")
                gate(
                    nc.sync.dma_start(
                        wt[:], wdT[j * P : (j + 1) * P, db * 512 : (db + 1) * 512]
                    ),
                    load_anchor[1],
                )
                wdt.append(wt)
            for dol in range(4):
                do = db * 4 + dol
                ps = mm_ps.tile([P, T], f32, name="d_ps", tag="mm")
                for j in range(32):
                    nc.tensor.matmul(
                        ps[:], wdt[j][:, dol * P : (dol + 1) * P], gu_tiles[j][:],
                        start=(j == 0), stop=(j == 31),
                    )
                o = pools["outp"].tile([P, T], f32, name="out_t", tag="out_t")
                nc.vector.tensor_scalar(
                    o[:], ps[:], c["bd"][:, do : do + 1], None, Alu.add
                )
                nc.vector.tensor_tensor(o[:], o[:], x1_tiles[do][:], Alu.add)
                nc.sync.dma_start(out[do * P : (do + 1) * P, :], o[:])


def _build():
    from contextlib import ExitStack
    from concourse import bass, mybir, tile, bacc

    f32 = mybir.dt.float32
    bf16 = mybir.dt.bfloat16

    nc = bacc.Bacc("TRN2", target_bir_lowering=False, num_devices=NCORES)

    xT = nc.declare_dram_parameter("xT", [D, T], f32, isOutput=False)
    wT = nc.declare_dram_parameter("wT", [D, 3 * D], bf16, isOutput=False)
    wpT = nc.declare_dram_parameter("wpT", [D, D], bf16, isOutput=False)
    wuT = nc.declare_dram_parameter("wuT", [D, DFF], bf16, isOutput=False)
    wdT = nc.declare_dram_parameter("wdT", [DFF, D], bf16, isOutput=False)
    bqk = nc.declare_dram_parameter("bqk", [P, 16], f32, isOutput=False)
    bv = nc.declare_dram_parameter("bv", [P, D], f32, isOutput=False)
    bp = nc.declare_dram_parameter("bp", [P, 8], f32, isOutput=False)
    bu = nc.declare_dram_parameter("bu", [P, 32], f32, isOutput=False)
    bd = nc.declare_dram_parameter("bd", [P, 8], f32, isOutput=False)
    tri = nc.declare_dram_parameter("tri", [CH, CH], bf16, isOutput=False)
    sel = nc.declare_dram_parameter("sel", [16, D], bf16, isOutput=False)
    out = nc.declare_dram_parameter("out", [D, T], f32, isOutput=True)

    a1i = {hb: nc.dram_tensor(f"a2a1_in{hb}", [NCORES * SLOT, CH], bf16) for hb in range(2)}
    a1o = {hb: nc.dram_tensor(f"a2a1_out{hb}", [NCORES * SLOT, CH], bf16) for hb in range(2)}
    a2i = {hb: nc.dram_tensor(f"a2a2_in{hb}", [NCORES * SLOT2, CH], bf16) for hb in range(2)}
    a2o = {hb: nc.dram_tensor(f"a2a2_out{hb}", [NCORES * SLOT2, CH], bf16) for hb in range(2)}

    with tile.TileContext(nc) as tc, ExitStack() as top:
        const = top.enter_context(tc.tile_pool(name="const", bufs=1))
        ones_d = const.tile([P, 1], bf16)
        nc.vector.memset(ones_d[:], 1.0 / D)
        ones_row = const.tile([1, P], bf16)
        nc.vector.memset(ones_row[:], 1.0)
        tri_t = [const.tile([P, CH], bf16, name=f"tri{s}", tag=f"tri{s}") for s in range(2)]
        for s in range(2):
            nc.sync.dma_start(tri_t[s][:], tri[s * P : (s + 1) * P, :])
        sel_t = const.tile([16, D], bf16, name="sel_t", tag="sel_t")
        nc.sync.dma_start(sel_t[:], sel[:, :])

        def ctile(name, param, shape):
            t = const.tile(shape, f32, name=name, tag=name)
            nc.sync.dma_start(t[:], param[:, :])
            return t

        consts = {
            "ones_d": ones_d, "ones_row": ones_row, "tri": tri_t, "sel": sel_t,
            "bqk": ctile("bqk_t", bqk, [P, 16]),
            "bv": ctile("bv_t", bv, [P, D]),
            "bp": ctile("bp_t", bp, [P, 8]),
            "bu": ctile("bu_t", bu, [P, 32]),
            "bd": ctile("bd_t", bd, [P, 8]),
        }

        pools = {
            "vec": top.enter_context(tc.tile_pool(name="vec", bufs=6)),
            "xt": top.enter_context(tc.tile_pool(name="xt", bufs=8)),
            "xba": top.enter_context(tc.tile_pool(name="xba", bufs=8)),
            "xbb": top.enter_context(tc.tile_pool(name="xbb", bufs=8)),
            "ht": top.enter_context(tc.tile_pool(name="ht", bufs=8)),
            "x1": top.enter_context(tc.tile_pool(name="x1", bufs=8)),
            "h2": top.enter_context(tc.tile_pool(name="h2", bufs=8)),
            "gu": top.enter_context(tc.tile_pool(name="gu", bufs=32)),
            "outp": top.enter_context(tc.tile_pool(name="outp", bufs=8)),
        }

        env = {
            "params": (xT, wT, wpT, wuT, wdT, out),
            "bounce": (a1i, a1o, a2i, a2o),
            "consts": consts,
            "pools": pools,
            "top": top,
        }
        _emit(nc, tc, env)

    nc.finalize()
    return nc


def _get_nc():
    if "nc" not in _CACHE:
        _CACHE["nc"] = _build()
    return _CACHE["nc"]


def _make_in_maps(inputs):
    x = np.asarray(inputs["x"], np.float32)
    ln1_g = np.asarray(inputs["ln1_g"], np.float32)
    ln1_b = np.asarray(inputs["ln1_b"], np.float32)
    W_attn = np.asarray(inputs["W_attn"], np.float32)
    b_attn = np.asarray(inputs["b_attn"], np.float32)
    W_proj = np.asarray(inputs["W_proj"], np.float32)
    b_proj = np.asarray(inputs["b_proj"], np.float32)
    ln2_g = np.asarray(inputs["ln2_g"], np.float32)
    ln2_b = np.asarray(inputs["ln2_b"], np.float32)
    W_up = np.asarray(inputs["W_up"], np.float32)
    b_up = np.asarray(inputs["b_up"], np.float32)
    W_down = np.asarray(inputs["W_down"], np.float32)
    b_down = np.asarray(inputs["b_down"], np.float32)

    bf = ml_dtypes.bfloat16

    # fold LN gamma/beta into the consuming weights/biases
    Wa = W_attn * ln1_g[None, :]
    ba = b_attn + W_attn @ ln1_b
    Wu = W_up * ln2_g[None, :]
    bu_ = b_up + W_up @ ln2_b

    wT = np.ascontiguousarray(Wa.T).astype(bf)
    wpT = np.ascontiguousarray(W_proj.T).astype(bf)
    wuT = np.ascontiguousarray(Wu.T).astype(bf)
    wdT = np.ascontiguousarray(W_down.T).astype(bf)

    def cols(v):  # [N] -> [128, N//128]: col j = v[j*128:(j+1)*128]
        return np.ascontiguousarray(v.reshape(-1, P).T).astype(np.float32)

    tri = np.tril(np.ones((CH, CH), np.float32)).T.astype(bf)  # tri[a,b] = a<=b
    tri = np.ascontiguousarray(tri)

    # sel[h, d] = 1 if head h owns output dim d (d//64 == h)
    sel = np.zeros((16, D), np.float32)
    for h in range(16):
        sel[h, h * DH : (h + 1) * DH] = 1.0
    sel = sel.astype(bf)

    common = dict(
        wT=wT, wpT=wpT, wuT=wuT, wdT=wdT, tri=tri, sel=sel,
        bqk=cols(ba[: 2 * D]),
        bv=np.ascontiguousarray(np.broadcast_to(ba[2 * D :].reshape(1, D), (P, D))).astype(np.float32),
        bp=cols(b_proj), bu=cols(bu_), bd=cols(b_down),
    )

    in_maps = []
    for i in range(NCORES):
        c0 = x[0, i * CH : (i + 1) * CH]  # [256, 1024]
        c1 = x[1, (7 - i) * CH : (8 - i) * CH]
        xTi = np.ascontiguousarray(np.concatenate([c0, c1], 0).T)  # [1024, 512]
        in_maps.append(dict(common, xT=xTi))
    return in_maps


def kernel(**inputs):
    in_maps = _make_in_maps(inputs)

    from concourse import bass_utils

    nc = _get_nc()
    res = bass_utils.run_bass_kernel_spmd(
        nc, in_maps, core_ids=list(range(NCORES)), trace=TRACE
    )
    _CACHE["last_res"] = res
    y = np.empty((B, S, D), np.float32)
    for i in range(NCORES):
        o = np.asarray(res.results[i]["out"], np.float32)  # [1024, 512]
        y[0, i * CH : (i + 1) * CH] = o[:, :CH].T
        y[1, (7 - i) * CH : (8 - i) * CH] = o[:, CH:].T
    return y


# revision 35
# speedup vs baseline: 1.0498x; 1.0498x over previous
"""Trainium2 distributed kernel for a dense transformer block (8 NeuronCores).

Sharding: tokens are data-parallel for LN/QKV/proj/MLP (512 tokens/core,
causal-balanced pairing: core i owns batch0 chunk i and batch1 chunk 7-i),
attention is head-parallel (2 heads/core) via AllToAll exchanges of Q/K/V.

v2 structure (vs v1):
  - Both AllToAlls are split into per-batch halves so they overlap compute
    (QKV for batch1 hides A2A#1a; attention on batch1 hides A2A#2a).
  - Softmax normalization is deferred: unnormalized O plus the denominator
    row ride A2A#2; the reciprocal+broadcast happens once per core after
    the exchange, off the attention critical path.
  - LayerNorm gamma/beta are folded into the weights host-side; rstd uses
    reciprocal_approx_fast; 1/D is folded into the stats ones-vector.
  - All copies/bias-adds are explicit nc.vector ops (scalar engine is
    reserved for Exp and Gelu).
"""

import sys

sys.path.insert(0, "/opt/trn_rl_repo")

import numpy as np
import ml_dtypes

NCORES = 8
D = 1024
H = 16
DH = 64
HL = H // NCORES  # heads per core = 2
B = 2
S = 2048
T = 512  # tokens per core
CH = 256  # token chunk (half of T = one batch's chunk)
DFF = 4096
P = 128
QR, KR, VR = 128, 128, 130  # slot row counts: qT, kT, packed-v regions
SLOT = QR + KR + VR  # 386
SLOT2 = 130  # a2 slot: 2 heads x (64 dims + 1 denom row)
EPS = 1e-5

_CACHE = {}
TRACE = False


def _emit(nc, tc, env):
    from contextlib import ExitStack

    from concourse import bass, mybir

    f32 = mybir.dt.float32
    bf16 = mybir.dt.bfloat16
    Alu = mybir.AluOpType
    AFT = mybir.ActivationFunctionType

    (xT, wT, wpT, wuT, wdT, out) = env["params"]
    (a1i, a1o, a2i, a2o) = env["bounce"]  # dicts {0: tensorA, 1: tensorB}
    c = env["consts"]
    pools = env["pools"]
    rg = [list(range(NCORES))]
    vec = pools["vec"]

    # ---- persistent psum pool (bank budget: mm 3; att pools are scoped) ----
    top = env["top"]
    mm_ps = top.enter_context(tc.tile_pool(name="mm_ps", bufs=3, space="PSUM"))

    def layer_norm_T(x_tiles, xb_tiles, out_pool, pfx):
        """x_tiles: 8x[128,T] (f32 or None), xb_tiles: 8x[128,T] bf16 casts
        (made here if None). Returns 8 normalized bf16 z tiles [128,T]."""
        with tc.tile_pool(name=f"lnst{pfx}", bufs=1, space="PSUM") as lnp, tc.tile_pool(
            name=f"lntmp{pfx}", bufs=3
        ) as tmp_p:
            if xb_tiles is None:
                xb_tiles = []
                for dk in range(8):
                    xb = pools[f"xb{pfx}"].tile([P, T], bf16, name="xb", tag="xb")
                    nc.scalar.activation(xb[:], x_tiles[dk][:], AFT.Copy)
                    xb_tiles.append(xb)
            ps_sum = lnp.tile([1, T], f32, name="ps_sum", tag="ps_sum")
            ps_sq = lnp.tile([1, T], f32, name="ps_sq", tag="ps_sq")
            for dk in range(8):
                nc.tensor.matmul(
                    ps_sum[:], c["ones_d"][:], xb_tiles[dk][:],
                    start=(dk == 0), stop=(dk == 7),
                )
                sq = tmp_p.tile([P, T], bf16, name="sq", tag="sq")
                nc.vector.tensor_tensor(sq[:], xb_tiles[dk][:], xb_tiles[dk][:], Alu.mult)
                nc.tensor.matmul(
                    ps_sq[:], c["ones_d"][:], sq[:], start=(dk == 0), stop=(dk == 7)
                )
            # mu = ps_sum (1/D folded in), msq = ps_sq
            mu_s = vec.tile([1, T], f32, name="mu_s", tag="lnvec")
            nc.vector.tensor_copy(mu_s[:], ps_sum[:])
            mu2 = vec.tile([1, T], f32, name="mu2", tag="lnvec")
            nc.vector.tensor_tensor(mu2[:], mu_s[:], mu_s[:], Alu.mult)
            var = vec.tile([1, T], f32, name="var", tag="lnvec")
            nc.vector.tensor_tensor(var[:], ps_sq[:], mu2[:], Alu.subtract)
            nc.vector.tensor_scalar(var[:], var[:], EPS, None, Alu.add)
            rvar = vec.tile([1, T], f32, name="rvar", tag="lnvec")
            nc.vector.reciprocal(rvar[:], var[:])
            rstd_c = vec.tile([1, T], bf16, name="rstd_c", tag="lnvec")
            nc.scalar.activation(rstd_c[:], rvar[:], AFT.Sqrt)
            mur_c = vec.tile([1, T], bf16, name="mur_c", tag="lnvec")
            with nc.allow_low_precision(reason="ln mean*rstd bcast"):
                nc.vector.tensor_tensor(mur_c[:], mu_s[:], rstd_c[:], Alu.mult)
            rstd_b = lnp.tile([P, T], f32, name="rstd_b", tag="rstd_b")
            nc.tensor.matmul(rstd_b[:], c["ones_row"][:], rstd_c[:], start=True, stop=True)
            mur_b = lnp.tile([P, T], f32, name="mur_b", tag="mur_b")
            nc.tensor.matmul(mur_b[:], c["ones_row"][:], mur_c[:], start=True, stop=True)
            # bf16 SBUF copies of the broadcasts so applies hit DVE 2-byte modes
            rstd_bs = tmp_p.tile([P, T], bf16, name="rstd_bs", tag="rstd_bs")
            nc.vector.tensor_copy(rstd_bs[:], rstd_b[:])
            mur_bs = tmp_p.tile([P, T], bf16, name="mur_bs", tag="mur_bs")
            nc.vector.tensor_copy(mur_bs[:], mur_b[:])
            outs = []
            for dk in range(8):
                t1 = tmp_p.tile([P, T], bf16, name="lnt1", tag="lnt1")
                nc.vector.tensor_tensor(t1[:], xb_tiles[dk][:], rstd_bs[:], Alu.mult)
                o = out_pool.tile([P, T], bf16, name="ln_out", tag="ln_out")
                nc.vector.tensor_tensor(o[:], t1[:], mur_bs[:], Alu.subtract)
                outs.append(o)
            return outs

    # ================= load x, LN1 =================
    x_tiles = []
    for dk in range(8):
        xt = pools["xt"].tile([P, T], f32, name="xt", tag="xt")
        nc.sync.dma_start(xt[:], xT[dk * P : (dk + 1) * P, :])
        x_tiles.append(xt)
    h_tiles = layer_norm_T(x_tiles, None, pools["ht"], "a")

    # ================= QKV per half + A2A#1 =================
    # half hb: token cols hb*CH..(hb+1)*CH of this core's T tokens
    att_scope = ExitStack()
    att_s = att_scope.enter_context(tc.tile_pool(name="att_s", bufs=3, space="PSUM"))
    kv_p = att_scope.enter_context(tc.tile_pool(name="kv", bufs=16))
    qe_p = att_scope.enter_context(tc.tile_pool(name="qe", bufs=8))
    loads = {}  # b -> [(k_ts, v_ts, q_ts) per lh]
    load_anchor = {}  # b -> last attention-input load DMA (gates weight prefetch)

    def emit_attn_loads(b):
        """Issue K/V/Q loads for batch b right after its A2A#1 so they sit at
        the DMA queue heads, ahead of weight prefetch and later staging."""
        a1x = a1o[b]
        per_lh = []
        for lh in range(HL):
            k_ts, v_ts, q_ts = [], [], []
            for kc in range(8):
                slot = kc if b == 0 else 7 - kc
                kt = kv_p.tile([DH, CH], bf16, name="kt", tag="kt")
                nc.sync.dma_start(
                    kt[:],
                    a1x[slot * SLOT + QR + lh * DH : slot * SLOT + QR + (lh + 1) * DH, :],
                )
                k_ts.append(kt)
                for sub in range(2):
                    vt = kv_p.tile([P, 65], bf16, name="vt", tag="vt", bufs=32)
                    off = (slot * SLOT + QR + KR) * CH + (sub * P) * VR + lh * 65
                    nc.sync.dma_start(vt[:], bass.AP(a1x, off, [[VR, P], [1, 65]]))
                    v_ts.append(vt)
            for pr in range(4):
                q0, q1 = 2 * pr, 2 * pr + 1
                s0 = q0 if b == 0 else 7 - q0
                s1 = q1 if b == 0 else 7 - q1
                qt = qe_p.tile([DH, 2 * CH], bf16, name="qt", tag="qt", bufs=16)
                nc.sync.dma_start(
                    qt[:, 0:CH],
                    a1x[s0 * SLOT + lh * DH : s0 * SLOT + (lh + 1) * DH, :],
                )
                load_anchor[b] = nc.sync.dma_start(
                    qt[:, CH : 2 * CH],
                    a1x[s1 * SLOT + lh * DH : s1 * SLOT + (lh + 1) * DH, :],
                )
                q_ts.append(qt)
            per_lh.append((k_ts, v_ts, q_ts))
        loads[b] = per_lh

    with tc.tile_pool(name="wqk", bufs=16) as wqk_p, tc.tile_pool(
        name="stg", bufs=8
    ) as stg_p, tc.tile_pool(name="vst", bufs=6) as vst_p:
        for hb in range(2):
            a1x = a1i[hb]
            cols = slice(hb * CH, (hb + 1) * CH)
            # q/k: 16 output blocks of 128 dims (weights re-loaded per half)
            for blk in range(4):
                wrow = []
                for dk in range(8):
                    wt = wqk_p.tile([P, 512], bf16, name="wqk", tag="wqk")
                    nc.sync.dma_start(
                        wt[:], wT[dk * P : (dk + 1) * P, blk * 512 : (blk + 1) * 512]
                    )
                    wrow.append(wt)
                for jl in range(4):
                    jt = blk * 4 + jl  # 0..15 (0-7 q, 8-15 k)
                    ps = mm_ps.tile([P, CH], f32, name="qk_ps", tag="mm")
                    for dk in range(8):
                        nc.tensor.matmul(
                            ps[:],
                            wrow[dk][:, jl * P : (jl + 1) * P],
                            h_tiles[dk][:, cols],
                            start=(dk == 0), stop=(dk == 7),
                        )
                    stg = stg_p.tile([P, CH], bf16, name="stg", tag="stg")
                    nc.vector.tensor_scalar(
                        stg[:], ps[:], c["bqk"][:, jt : jt + 1], None, Alu.add
                    )
                    r0 = jt * SLOT if jt < 8 else (jt - 8) * SLOT + QR
                    nc.sync.dma_start(a1x[r0 : r0 + P, :], stg[:])
            # v: out [128 tok, 512 vdim], token chunks tt within this half
            for jc in range(2):
                wvrow = []
                for dk in range(8):
                    wt = wqk_p.tile([P, 512], bf16, name="wv", tag="wv", bufs=16)
                    nc.sync.dma_start(
                        wt[:],
                        wT[dk * P : (dk + 1) * P, 2048 + jc * 512 : 2048 + (jc + 1) * 512],
                    )
                    wvrow.append(wt)
                for tt in range(2):
                    ps = mm_ps.tile([P, 512], f32, name="v_ps", tag="mm")
                    t0 = hb * CH + tt * P
                    for dk in range(8):
                        nc.tensor.matmul(
                            ps[:], h_tiles[dk][:, t0 : t0 + P], wvrow[dk][:],
                            start=(dk == 0), stop=(dk == 7),
                        )
                    for sl in range(4):
                        slot = jc * 4 + sl
                        vt = vst_p.tile([P, VR], bf16, name="vst", tag="vst")
                        for lh in range(HL):
                            cc = slot * P + lh * DH - jc * 512
                            nc.vector.tensor_tensor(
                                vt[:, lh * 65 : lh * 65 + DH],
                                ps[:, cc : cc + DH],
                                c["bv"][:, slot * P + lh * DH : slot * P + lh * DH + DH],
                                Alu.add,
                            )
                            nc.vector.memset(vt[:, lh * 65 + DH : lh * 65 + DH + 1], 1.0)
                        off = (slot * SLOT + QR + KR) * CH + (tt * P) * VR
                        dst = bass.AP(a1x, off, [[VR, P], [1, 65 * HL]])
                        nc.sync.dma_start(dst, vt[:])
            nc.gpsimd.collective_compute(
                "AllToAll", mybir.AluOpType.bypass, replica_groups=rg,
                ins=[a1x.ap().opt()], outs=[a1o[hb].ap().opt()],
            )
            emit_attn_loads(hb)

    # ================= attention per batch + A2A#2 =================
    with tc.tile_pool(name="att_o", bufs=2, space="PSUM") as att_o, tc.tile_pool(
        name="a2stg", bufs=16
    ) as a2s_p:
        for b in range(B):
            a2x = a2i[b]
            stage = [
                [
                    a2s_p.tile([65, CH], bf16, name=f"a2stg{b}_{j}_{lh}", tag="a2stg", bufs=32)
                    for lh in range(HL)
                ]
                for j in range(8)
            ]
            for lh in range(HL):
                k_ts, v_ts, q_ts = loads[b][lh]
                for pr in range(4):  # query-chunk pairs (2pr, 2pr+1)
                    q0, q1 = 2 * pr, 2 * pr + 1
                    s0 = q0 if b == 0 else 7 - q0
                    s1 = q1 if b == 0 else 7 - q1
                    qt = q_ts[pr]
                    po = att_o.tile([65, 2 * CH], f32, name="o_ps", tag="o")
                    n_mm = 2 * (q1 + 1)
                    mi = 0
                    for kc in range(q1 + 1):
                        for sub in range(2):
                            ps = att_s.tile([P, 2 * CH], f32, name="s_ps", tag="s")
                            nc.tensor.matmul(
                                ps[:], k_ts[kc][:, sub * P : (sub + 1) * P], qt[:],
                                start=True, stop=True,
                            )
                            E = qe_p.tile([P, 2 * CH], bf16, name="E", tag="E")
                            if kc == q1:
                                # future keys for q0-half: zero; diagonal for q1-half
                                nc.scalar.activation(
                                    E[:, CH : 2 * CH], ps[:, CH : 2 * CH],
                                    AFT.Exp, scale=0.125,
                                )
                                nc.vector.memset(E[:, 0:CH], 0.0)
                                nc.vector.tensor_tensor(
                                    E[:, CH : 2 * CH], E[:, CH : 2 * CH],
                                    c["tri"][sub][:], Alu.mult,
                                )
                            else:
                                nc.scalar.activation(E[:], ps[:], AFT.Exp, scale=0.125)
                                if kc == q0:  # diagonal for q0-half only
                                    nc.vector.tensor_tensor(
                                        E[:, 0:CH], E[:, 0:CH], c["tri"][sub][:], Alu.mult
                                    )
                            nc.tensor.matmul(
                                po[:], v_ts[kc * 2 + sub][:], E[:],
                                start=(mi == 0), stop=(mi == n_mm - 1),
                            )
                            mi += 1
                    # stage unnormalized O + denominator row (row 64)
                    for half, sq in ((0, s0), (1, s1)):
                        nc.vector.tensor_copy(
                            stage[sq][lh][:],
                            po[:, half * CH : (half + 1) * CH],
                        )
            for j in range(8):
                for lh in range(HL):
                    nc.sync.dma_start(
                        a2x[j * SLOT2 + lh * 65 : j * SLOT2 + (lh + 1) * 65, :],
                        stage[j][lh][:],
                    )
            nc.gpsimd.collective_compute(
                "AllToAll", mybir.AluOpType.bypass, replica_groups=rg,
                ins=[a2x.ap().opt()], outs=[a2o[b].ap().opt()],
            )

    # ================= post-norm + proj + residual1 =================
    from concourse.bass import _add_dep_helper

    def gate(dma_inst, anchor):
        """Order a weight-prefetch DMA after the attention input loads so it
        doesn't hog the DMA queues during the collective window."""
        _add_dep_helper(
            dma_inst.ins, anchor.ins, sync=False, reason="defer weight prefetch"
        )

    x1_tiles = []
    with tc.tile_pool(name="wp", bufs=16) as wp_p, tc.tile_pool(
        name="onrm", bufs=6
    ) as onrm_p, tc.tile_pool(name="obf", bufs=8) as obf_p:
        wpt = {}
        for dk in range(8):
            for db in range(2):
                wt = wp_p.tile([P, 512], bf16, name="wp", tag="wp")
                gate(
                    nc.sync.dma_start(
                        wt[:], wpT[dk * P : (dk + 1) * P, db * 512 : (db + 1) * 512]
                    ),
                    load_anchor[0],
                )
                wpt[(dk, db)] = wt
        o_bf = []
        for dk in range(8):
            t = obf_p.tile([P, T], bf16, name="obf", tag="obf")
            o_bf.append(t)
        for hb in range(2):
            a2x = a2o[hb]
            # denominators: [16, CH] (core c' rows lh*65+64)
            den_bf = onrm_p.tile([16, CH], bf16, name="den_bf", tag="den_bf")
            dsrc = bass.AP(a2x, DH * CH, [[SLOT2 * CH, 8], [65 * CH, 2], [1, CH]])
            nc.sync.dma_start(den_bf[:], dsrc)
            den = onrm_p.tile([16, CH], f32, name="den", tag="den")
            nc.vector.tensor_copy(den[:], den_bf[:])
            rec = onrm_p.tile([16, CH], f32, name="rec", tag="den")
            nc.vector.reciprocal(rec[:], den[:])
            rec_c = onrm_p.tile([16, CH], bf16, name="rec_c", tag="den_bf")
            with nc.allow_low_precision(reason="softmax denom bcast"):
                nc.vector.tensor_copy(rec_c[:], rec[:])
            for dk in range(8):
                o_un = onrm_p.tile([P, CH], bf16, name="o_un", tag="o_un")
                osrc = bass.AP(a2x, dk * SLOT2 * CH, [[65 * CH, 2], [CH, DH], [1, CH]])
                nc.sync.dma_start(o_un[:], osrc)
                rec_b = att_s.tile([P, CH], f32, name="rec_b", tag="s")
                nc.tensor.matmul(
                    rec_b[:], c["sel"][:, dk * P : (dk + 1) * P], rec_c[:],
                    start=True, stop=True,
                )
                nc.vector.tensor_tensor(
                    o_bf[dk][:, hb * CH : (hb + 1) * CH], o_un[:], rec_b[:], Alu.mult
                )
        x1_tiles = [
            pools["x1"].tile([P, T], f32, name="x1", tag="x1") for _ in range(8)
        ]
        for hb in range(2):  # proj per half so half A overlaps A2A#2B
            cols = slice(hb * CH, (hb + 1) * CH)
            for do in range(8):
                ps = mm_ps.tile([P, CH], f32, name="p_ps", tag="mm")
                for dk in range(8):
                    nc.tensor.matmul(
                        ps[:],
                        wpt[(dk, do // 4)][:, (do % 4) * P : (do % 4 + 1) * P],
                        o_bf[dk][:, cols],
                        start=(dk == 0), stop=(dk == 7),
                    )
                x1 = x1_tiles[do]
                nc.vector.tensor_scalar(
                    x1[:, cols], ps[:], c["bp"][:, do : do + 1], None, Alu.add
                )
                nc.vector.tensor_tensor(
                    x1[:, cols], x1[:, cols], x_tiles[do][:, cols], Alu.add
                )
    att_scope.close()  # frees att_s psum banks before LN2 opens its pool

    # ================= LN2 =================
    h2_tiles = layer_norm_T(x1_tiles, None, pools["h2"], "b")

    # ================= MLP up + gelu =================
    gu_tiles = []
    with tc.tile_pool(name="wu", bufs=16) as wu_p:
        for jb in range(8):
            wut = []
            for dk in range(8):
                wt = wu_p.tile([P, 512], bf16, name="wu", tag="wu")
                gate(
                    nc.sync.dma_start(
                        wt[:], wuT[dk * P : (dk + 1) * P, jb * 512 : (jb + 1) * 512]
                    ),
                    load_anchor[1],
                )
                wut.append(wt)
            for jl in range(4):
                j = jb * 4 + jl
                ps = mm_ps.tile([P, T], f32, name="u_ps", tag="mm")
                for dk in range(8):
                    nc.tensor.matmul(
                        ps[:], wut[dk][:, jl * P : (jl + 1) * P], h2_tiles[dk][:],
                        start=(dk == 0), stop=(dk == 7),
                    )
                gu = pools["gu"].tile([P, T], bf16, name="gu", tag="gu")
                nc.scalar.activation(
                    gu[:], ps[:], AFT.Gelu_apprx_tanh, bias=c["bu"][:, j : j + 1]
                )
                gu_tiles.append(gu)

    # ================= MLP down + residual2 =================
    with tc.tile_pool(name="wd", bufs=34) as wd_p:
        for db in range(2):
            wdt = []
            for j in range(32):
                wt = wd_p.tile([P, 512], bf16, name="wd", tag="wd")
                gate(
                    nc.sync.dma_start(
                        wt[:], wdT[j * P : (j + 1) * P, db * 512 : (db + 1) * 512]
                    ),
                    load_anchor[1],
                )
                wdt.append(wt)
            for dol in range(4):
                do = db * 4 + dol
                ps = mm_ps.tile([P, T], f32, name="d_ps", tag="mm")
                for j in range(32):
                    nc.tensor.matmul(
                        ps[:], wdt[j][:, dol * P : (dol + 1) * P], gu_tiles[j][:],
                        start=(j == 0), stop=(j == 31),
                    )
                o = pools["outp"].tile([P, T], f32, name="out_t", tag="out_t")
                nc.vector.tensor_scalar(
                    o[:], ps[:], c["bd"][:, do : do + 1], None, Alu.add
                )
                nc.vector.tensor_tensor(o[:], o[:], x1_tiles[do][:], Alu.add)
                nc.sync.dma_start(out[do * P : (do + 1) * P, :], o[:])


def _build():
    from contextlib import ExitStack
    from concourse import bass, mybir, tile, bacc

    f32 = mybir.dt.float32
    bf16 = mybir.dt.bfloat16

    nc = bacc.Bacc("TRN2", target_bir_lowering=False, num_devices=NCORES)

    xT = nc.declare_dram_parameter("xT", [D, T], f32, isOutput=False)
    wT = nc.declare_dram_parameter("wT", [D, 3 * D], bf16, isOutput=False)
    wpT = nc.declare_dram_parameter("wpT", [D, D], bf16, isOutput=False)
    wuT = nc.declare_dram_parameter("wuT", [D, DFF], bf16, isOutput=False)
    wdT = nc.declare_dram_parameter("wdT", [DFF, D], bf16, isOutput=False)
    bqk = nc.declare_dram_parameter("bqk", [P, 16], f32, isOutput=False)
    bv = nc.declare_dram_parameter("bv", [P, D], f32, isOutput=False)
    bp = nc.declare_dram_parameter("bp", [P, 8], f32, isOutput=False)
    bu = nc.declare_dram_parameter("bu", [P, 32], f32, isOutput=False)
    bd = nc.declare_dram_parameter("bd", [P, 8], f32, isOutput=False)
    tri = nc.declare_dram_parameter("tri", [CH, CH], bf16, isOutput=False)
    sel = nc.declare_dram_parameter("sel", [16, D], bf16, isOutput=False)
    out = nc.declare_dram_parameter("out", [D, T], f32, isOutput=True)

    a1i = {hb: nc.dram_tensor(f"a2a1_in{hb}", [NCORES * SLOT, CH], bf16) for hb in range(2)}
    a1o = {hb: nc.dram_tensor(f"a2a1_out{hb}", [NCORES * SLOT, CH], bf16) for hb in range(2)}
    a2i = {hb: nc.dram_tensor(f"a2a2_in{hb}", [NCORES * SLOT2, CH], bf16) for hb in range(2)}
    a2o = {hb: nc.dram_tensor(f"a2a2_out{hb}", [NCORES * SLOT2, CH], bf16) for hb in range(2)}

    with tile.TileContext(nc) as tc, ExitStack() as top:
        const = top.enter_context(tc.tile_pool(name="const", bufs=1))
        ones_d = const.tile([P, 1], bf16)
        nc.vector.memset(ones_d[:], 1.0 / D)
        ones_row = const.tile([1, P], bf16)
        nc.vector.memset(ones_row[:], 1.0)
        tri_t = [const.tile([P, CH], bf16, name=f"tri{s}", tag=f"tri{s}") for s in range(2)]
        for s in range(2):
            nc.sync.dma_start(tri_t[s][:], tri[s * P : (s + 1) * P, :])
        sel_t = const.tile([16, D], bf16, name="sel_t", tag="sel_t")
        nc.sync.dma_start(sel_t[:], sel[:, :])

        def ctile(name, param, shape):
            t = const.tile(shape, f32, name=name, tag=name)
            nc.sync.dma_start(t[:], param[:, :])
            return t

        consts = {
            "ones_d": ones_d, "ones_row": ones_row, "tri": tri_t, "sel": sel_t,
            "bqk": ctile("bqk_t", bqk, [P, 16]),
            "bv": ctile("bv_t", bv, [P, D]),
            "bp": ctile("bp_t", bp, [P, 8]),
            "bu": ctile("bu_t", bu, [P, 32]),
            "bd": ctile("bd_t", bd, [P, 8]),
        }

        pools = {
            "vec": top.enter_context(tc.tile_pool(name="vec", bufs=4)),
            "xt": top.enter_context(tc.tile_pool(name="xt", bufs=8)),
            "xba": top.enter_context(tc.tile_pool(name="xba", bufs=8)),
            "xbb": top.enter_context(tc.tile_pool(name="xbb", bufs=8)),
            "ht": top.enter_context(tc.tile_pool(name="ht", bufs=8)),
            "x1": top.enter_context(tc.tile_pool(name="x1", bufs=8)),
            "h2": top.enter_context(tc.tile_pool(name="h2", bufs=8)),
            "gu": top.enter_context(tc.tile_pool(name="gu", bufs=32)),
            "outp": top.enter_context(tc.tile_pool(name="outp", bufs=4)),
        }

        env = {
            "params": (xT, wT, wpT, wuT, wdT, out),
            "bounce": (a1i, a1o, a2i, a2o),
            "consts": consts,
            "pools": pools,
            "top": top,
        }
        _emit(nc, tc, env)

    nc.finalize()
    return nc


def _get_nc():
    if "nc" not in _CACHE:
        _CACHE["nc"] = _build()
    return _CACHE["nc"]


def _make_in_maps(inputs):
    x = np.asarray(inputs["x"], np.float32)
    ln1_g = np.asarray(inputs["ln1_g"], np.float32)
    ln1_b = np.asarray(inputs["ln1_b"], np.float32)
    W_attn = np.asarray(inputs["W_attn"], np.float32)
    b_attn = np.asarray(inputs["b_attn"], np.float32)
    W_proj = np.asarray(inputs["W_proj"], np.float32)
    b_proj = np.asarray(inputs["b_proj"], np.float32)
    ln2_g = np.asarray(inputs["ln2_g"], np.float32)
    ln2_b = np.asarray(inputs["ln2_b"], np.float32)
    W_up = np.asarray(inputs["W_up"], np.float32)
    b_up = np.asarray(inputs["b_up"], np.float32)
    W_down = np.asarray(inputs["W_down"], np.float32)
    b_down = np.asarray(inputs["b_down"], np.float32)

    bf = ml_dtypes.bfloat16

    # fold LN gamma/beta into the consuming weights/biases
    Wa = W_attn * ln1_g[None, :]
    ba = b_attn + W_attn @ ln1_b
    Wu = W_up * ln2_g[None, :]
    bu_ = b_up + W_up @ ln2_b

    wT = np.ascontiguousarray(Wa.T).astype(bf)
    wpT = np.ascontiguousarray(W_proj.T).astype(bf)
    wuT = np.ascontiguousarray(Wu.T).astype(bf)
    wdT = np.ascontiguousarray(W_down.T).astype(bf)

    def cols(v):  # [N] -> [128, N//128]: col j = v[j*128:(j+1)*128]
        return np.ascontiguousarray(v.reshape(-1, P).T).astype(np.float32)

    tri = np.tril(np.ones((CH, CH), np.float32)).T.astype(bf)  # tri[a,b] = a<=b
    tri = np.ascontiguousarray(tri)

    # sel[h, d] = 1 if head h owns output dim d (d//64 == h)
    sel = np.zeros((16, D), np.float32)
    for h in range(16):
        sel[h, h * DH : (h + 1) * DH] = 1.0
    sel = sel.astype(bf)

    common = dict(
        wT=wT, wpT=wpT, wuT=wuT, wdT=wdT, tri=tri, sel=sel,
        bqk=cols(ba[: 2 * D]),
        bv=np.ascontiguousarray(np.broadcast_to(ba[2 * D :].reshape(1, D), (P, D))).astype(np.float32),
        bp=cols(b_proj), bu=cols(bu_), bd=cols(b_down),
    )

    in_maps = []
    for i in range(NCORES):
        c0 = x[0, i * CH : (i + 1) * CH]  # [256, 1024]
        c1 = x[1, (7 - i) * CH : (8 - i) * CH]
        xTi = np.ascontiguousarray(np.concatenate([c0, c1], 0).T)  # [1024, 512]
        in_maps.append(dict(common, xT=xTi))
    return in_maps


def kernel(**inputs):
    in_maps = _make_in_maps(inputs)

    from concourse import bass_utils

    nc = _get_nc()
    res = bass_utils.run_bass_kernel_spmd(
        nc, in_maps, core_ids=list(range(NCORES)), trace=TRACE
    )
    _CACHE["last_res"] = res
    y = np.empty((B, S, D), np.float32)
    for i in range(NCORES):
        o = np.asarray(res.results[i]["out"], np.float32)  # [1024, 512]
        y[0, i * CH : (i + 1) * CH] = o[:, :CH].T
        y[1, (7 - i) * CH : (8 - i) * CH] = o[:, CH:].T
    return y


# revision 45
# speedup vs baseline: 1.1159x; 1.0629x over previous
"""Trainium2 distributed kernel for a dense transformer block (8 NeuronCores).

Sharding: tokens are data-parallel for LN/QKV/proj/MLP (512 tokens/core,
causal-balanced pairing: core i owns batch0 chunk i and batch1 chunk 7-i),
attention is head-parallel (2 heads/core) via AllToAll exchanges of Q/K/V.

v2 structure (vs v1):
  - Both AllToAlls are split into per-batch halves so they overlap compute
    (QKV for batch1 hides A2A#1a; attention on batch1 hides A2A#2a).
  - Softmax normalization is deferred: unnormalized O plus the denominator
    row ride A2A#2; the reciprocal+broadcast happens once per core after
    the exchange, off the attention critical path.
  - LayerNorm gamma/beta are folded into the weights host-side; rstd uses
    reciprocal_approx_fast; 1/D is folded into the stats ones-vector.
  - All copies/bias-adds are explicit nc.vector ops (scalar engine is
    reserved for Exp and Gelu).
"""

import sys

sys.path.insert(0, "/opt/trn_rl_repo")

import numpy as np
import ml_dtypes

NCORES = 8
D = 1024
H = 16
DH = 64
HL = H // NCORES  # heads per core = 2
B = 2
S = 2048
T = 512  # tokens per core
CH = 256  # token chunk (half of T = one batch's chunk)
DFF = 4096
P = 128
QR, KR, VR = 128, 128, 130  # slot row counts: qT, kT, packed-v regions
SLOT = QR + KR + VR  # 386
SLOT2 = 130  # a2 slot: 2 heads x (64 dims + 1 denom row)
EPS = 1e-5

_CACHE = {}
TRACE = False


def _emit(nc, tc, env):
    from contextlib import ExitStack

    from concourse import bass, mybir

    f32 = mybir.dt.float32
    bf16 = mybir.dt.bfloat16
    Alu = mybir.AluOpType
    AFT = mybir.ActivationFunctionType

    (xT, wT, wpT, wuT, wdT, out) = env["params"]
    (a1i, a1o, a2i, a2o) = env["bounce"]  # dicts {0: tensorA, 1: tensorB}
    c = env["consts"]
    pools = env["pools"]
    rg = [list(range(NCORES))]
    vec = pools["vec"]

    # ---- persistent psum pool (bank budget: mm 3; att pools are scoped) ----
    top = env["top"]
    mm_ps = top.enter_context(tc.tile_pool(name="mm_ps", bufs=3, space="PSUM"))

    def layer_norm_T(x_tiles, xb_tiles, out_pool, pfx):
        """x_tiles: 8x[128,T] (f32 or None), xb_tiles: 8x[128,T] bf16 casts
        (made here if None). Returns 8 normalized bf16 z tiles [128,T]."""
        with tc.tile_pool(name=f"lnst{pfx}", bufs=1, space="PSUM") as lnp, tc.tile_pool(
            name=f"lntmp{pfx}", bufs=3
        ) as tmp_p:
            if xb_tiles is None:
                xb_tiles = []
                for dk in range(8):
                    xb = pools[f"xb{pfx}"].tile([P, T], bf16, name="xb", tag="xb")
                    nc.scalar.activation(xb[:], x_tiles[dk][:], AFT.Copy)
                    xb_tiles.append(xb)
            ps_sum = lnp.tile([1, T], f32, name="ps_sum", tag="ps_sum")
            ps_sq = lnp.tile([1, T], f32, name="ps_sq", tag="ps_sq")
            for dk in range(8):
                nc.tensor.matmul(
                    ps_sum[:], c["ones_d"][:], xb_tiles[dk][:],
                    start=(dk == 0), stop=(dk == 7),
                )
                sq = tmp_p.tile([P, T], bf16, name="sq", tag="sq")
                nc.vector.tensor_tensor(sq[:], xb_tiles[dk][:], xb_tiles[dk][:], Alu.mult)
                nc.tensor.matmul(
                    ps_sq[:], c["ones_d"][:], sq[:], start=(dk == 0), stop=(dk == 7)
                )
            # mu = ps_sum (1/D folded in), msq = ps_sq
            mu_s = vec.tile([1, T], f32, name="mu_s", tag="lnvec")
            nc.vector.tensor_copy(mu_s[:], ps_sum[:])
            mu2 = vec.tile([1, T], f32, name="mu2", tag="lnvec")
            nc.vector.tensor_tensor(mu2[:], mu_s[:], mu_s[:], Alu.mult)
            var = vec.tile([1, T], f32, name="var", tag="lnvec")
            nc.vector.tensor_tensor(var[:], ps_sq[:], mu2[:], Alu.subtract)
            nc.vector.tensor_scalar(var[:], var[:], EPS, None, Alu.add)
            rvar = vec.tile([1, T], f32, name="rvar", tag="lnvec")
            nc.vector.reciprocal(rvar[:], var[:])
            rstd_c = vec.tile([1, T], bf16, name="rstd_c", tag="lnvec")
            nc.scalar.activation(rstd_c[:], rvar[:], AFT.Sqrt)
            mur_c = vec.tile([1, T], bf16, name="mur_c", tag="lnvec")
            with nc.allow_low_precision(reason="ln mean*rstd bcast"):
                nc.vector.tensor_tensor(mur_c[:], mu_s[:], rstd_c[:], Alu.mult)
            rstd_b = lnp.tile([P, T], f32, name="rstd_b", tag="rstd_b")
            nc.tensor.matmul(rstd_b[:], c["ones_row"][:], rstd_c[:], start=True, stop=True)
            mur_b = lnp.tile([P, T], f32, name="mur_b", tag="mur_b")
            nc.tensor.matmul(mur_b[:], c["ones_row"][:], mur_c[:], start=True, stop=True)
            # bf16 SBUF copies of the broadcasts so applies hit DVE 2-byte modes
            rstd_bs = tmp_p.tile([P, T], bf16, name="rstd_bs", tag="rstd_bs")
            nc.vector.tensor_copy(rstd_bs[:], rstd_b[:])
            mur_bs = tmp_p.tile([P, T], bf16, name="mur_bs", tag="mur_bs")
            nc.vector.tensor_copy(mur_bs[:], mur_b[:])
            outs = []
            for dk in range(8):
                t1 = tmp_p.tile([P, T], bf16, name="lnt1", tag="lnt1")
                nc.vector.tensor_tensor(t1[:], xb_tiles[dk][:], rstd_bs[:], Alu.mult)
                o = out_pool.tile([P, T], bf16, name="ln_out", tag="ln_out")
                nc.vector.tensor_tensor(o[:], t1[:], mur_bs[:], Alu.subtract)
                outs.append(o)
            return outs

    # ================= load x, LN1 =================
    x_tiles = []
    for dk in range(8):
        xt = pools["xt"].tile([P, T], f32, name="xt", tag="xt")
        nc.sync.dma_start(xt[:], xT[dk * P : (dk + 1) * P, :])
        x_tiles.append(xt)
    h_tiles = layer_norm_T(x_tiles, None, pools["ht"], "a")

    # ================= QKV per half + A2A#1 =================
    # half hb: token cols hb*CH..(hb+1)*CH of this core's T tokens
    att_scope = ExitStack()
    att_s = att_scope.enter_context(tc.tile_pool(name="att_s", bufs=3, space="PSUM"))
    kv_p = att_scope.enter_context(tc.tile_pool(name="kv", bufs=16))
    qe_p = att_scope.enter_context(tc.tile_pool(name="qe", bufs=8))
    loads = {}  # b -> [(k_ts, v_ts, q_ts) per lh]
    load_anchor = {}  # b -> last attention-input load DMA (gates weight prefetch)

    def emit_attn_loads(b):
        """Batched K/V/Q loads for batch b, issued right after its A2A#1 on
        otherwise-idle sequencers (each dma_start costs ~0.6us of issue time).
        K: 4 paired loads [64, 2CH] per lh; V: 16 loads [128,130] covering both
        heads; Q: one 3D load per pr for b=0, two for b=1 (descending slots)."""
        a1x = a1o[b]
        eng = nc.scalar if b == 0 else nc.gpsimd
        v_ts = []
        for kc in range(8):
            slot = kc if b == 0 else 7 - kc
            for sub in range(2):
                vt = kv_p.tile([P, 2 * 65], bf16, name="vt", tag="vt", bufs=32)
                off = (slot * SLOT + QR + KR) * CH + (sub * P) * VR
                nc.sync.dma_start(vt[:], bass.AP(a1x, off, [[VR, P], [1, 2 * 65]]))
                v_ts.append(vt)
        kp = {}
        q_ts = {}
        for lh in range(HL):
            kp[lh] = []
            for p4 in range(4):
                kt = kv_p.tile([DH, 2 * CH], bf16, name="kt", tag="kt")
                lo_slot = 2 * p4 if b == 0 else 6 - 2 * p4
                src = bass.AP(
                    a1x,
                    (lo_slot * SLOT + QR + lh * DH) * CH,
                    [[CH, DH], [SLOT * CH, 2], [1, CH]],
                )
                eng.dma_start(kt[:], src)
                kp[lh].append(kt)
            q_ts[lh] = []
            for pr in range(4):
                q0, q1 = 2 * pr, 2 * pr + 1
                s0 = q0 if b == 0 else 7 - q0
                s1 = q1 if b == 0 else 7 - q1
                qt = qe_p.tile([DH, 2 * CH], bf16, name="qt", tag="qt", bufs=16)
                if b == 0:  # s1 == s0 + 1: single 3D load
                    src = bass.AP(
                        a1x,
                        (s0 * SLOT + lh * DH) * CH,
                        [[CH, DH], [SLOT * CH, 2], [1, CH]],
                    )
                    load_anchor[b] = eng.dma_start(qt[:], src)
                else:
                    eng.dma_start(
                        qt[:, 0:CH],
                        a1x[s0 * SLOT + lh * DH : s0 * SLOT + (lh + 1) * DH, :],
                    )
                    load_anchor[b] = eng.dma_start(
                        qt[:, CH : 2 * CH],
                        a1x[s1 * SLOT + lh * DH : s1 * SLOT + (lh + 1) * DH, :],
                    )
                q_ts[lh].append(qt)
        loads[b] = {"kp": kp, "v": v_ts, "q": q_ts}

    with tc.tile_pool(name="wqk", bufs=10) as wqk_p, tc.tile_pool(
        name="stg", bufs=6
    ) as stg_p, tc.tile_pool(name="vst", bufs=6) as vst_p:
        for hb in range(2):
            a1x = a1i[hb]
            cols = slice(hb * CH, (hb + 1) * CH)
            # q/k: 16 output blocks of 128 dims (weights re-loaded per half,
            # [P,1024] paired-block loads; staging paired into [P,2CH] writes)
            for bp in range(2):
                wrow = []
                for dk in range(8):
                    wt = wqk_p.tile([P, 1024], bf16, name="wqk", tag="wqk")
                    nc.sync.dma_start(
                        wt[:],
                        wT[dk * P : (dk + 1) * P, bp * 1024 : (bp + 1) * 1024],
                    )
                    wrow.append(wt)
                for blkl in range(2):
                    blk = bp * 2 + blkl
                    for jp in range(2):  # jt pairs (blk*4+2jp, +1)
                        jt0 = blk * 4 + 2 * jp
                        stg = stg_p.tile([P, 2 * CH], bf16, name="stg", tag="stg")
                        for jh in range(2):
                            jt = jt0 + jh
                            jl = jt % 4
                            ps = mm_ps.tile([P, CH], f32, name="qk_ps", tag="mm")
                            for dk in range(8):
                                nc.tensor.matmul(
                                    ps[:],
                                    wrow[dk][:, blkl * 512 + jl * P : blkl * 512 + (jl + 1) * P],
                                    h_tiles[dk][:, cols],
                                    start=(dk == 0), stop=(dk == 7),
                                )
                            nc.vector.tensor_scalar(
                                stg[:, jh * CH : (jh + 1) * CH], ps[:],
                                c["bqk"][:, jt : jt + 1], None, Alu.add,
                            )
                        r0 = jt0 * SLOT if jt0 < 8 else (jt0 - 8) * SLOT + QR
                        dst = bass.AP(
                            a1x, r0 * CH, [[CH, P], [SLOT * CH, 2], [1, CH]]
                        )
                        nc.sync.dma_start(dst, stg[:])
            # v: out [128 tok, 512 vdim], token chunks tt within this half
            wvrow = []
            for dk in range(8):
                wt = wqk_p.tile([P, 1024], bf16, name="wv", tag="wv", bufs=8)
                nc.sync.dma_start(
                    wt[:], wT[dk * P : (dk + 1) * P, 2048 : 2048 + 1024]
                )
                wvrow.append(wt)
            for jc in range(2):
                for tt in range(2):
                    ps = mm_ps.tile([P, 512], f32, name="v_ps", tag="mm")
                    t0 = hb * CH + tt * P
                    for dk in range(8):
                        nc.tensor.matmul(
                            ps[:],
                            h_tiles[dk][:, t0 : t0 + P],
                            wvrow[dk][:, jc * 512 : (jc + 1) * 512],
                            start=(dk == 0), stop=(dk == 7),
                        )
                    for sp in range(2):  # slot pairs
                        slot0 = jc * 4 + 2 * sp
                        vt = vst_p.tile([P, 2 * VR], bf16, name="vst", tag="vst")
                        for sh in range(2):
                            slot = slot0 + sh
                            for lh in range(HL):
                                cc = slot * P + lh * DH - jc * 512
                                col = sh * VR + lh * 65
                                nc.vector.tensor_tensor(
                                    vt[:, col : col + DH],
                                    ps[:, cc : cc + DH],
                                    c["bv"][:, slot * P + lh * DH : slot * P + lh * DH + DH],
                                    Alu.add,
                                )
                                nc.vector.memset(vt[:, col + DH : col + DH + 1], 1.0)
                        off = (slot0 * SLOT + QR + KR) * CH + (tt * P) * VR
                        dst = bass.AP(
                            a1x, off, [[VR, P], [SLOT * CH, 2], [1, VR]]
                        )
                        nc.sync.dma_start(dst, vt[:])
            nc.gpsimd.collective_compute(
                "AllToAll", mybir.AluOpType.bypass, replica_groups=rg,
                ins=[a1x.ap().opt()], outs=[a1o[hb].ap().opt()],
            )
            emit_attn_loads(hb)

    # ================= attention per batch + A2A#2 =================
    with tc.tile_pool(name="att_o", bufs=2, space="PSUM") as att_o, tc.tile_pool(
        name="a2stg", bufs=16
    ) as a2s_p:
        for b in range(B):
            a2x = a2i[b]
            stage = [
                a2s_p.tile([65, 2 * CH], bf16, name=f"a2stg{b}_{j}", tag="a2stg")
                for j in range(8)
            ]
            for lh in range(HL):
                kp = loads[b]["kp"][lh]
                v_ts = loads[b]["v"]
                q_ts = loads[b]["q"][lh]
                for pr in range(4):  # query-chunk pairs (2pr, 2pr+1)
                    q0, q1 = 2 * pr, 2 * pr + 1
                    s0 = q0 if b == 0 else 7 - q0
                    s1 = q1 if b == 0 else 7 - q1
                    qt = q_ts[pr]
                    po = att_o.tile([65, 2 * CH], f32, name="o_ps", tag="o")
                    n_mm = 2 * (q1 + 1)
                    mi = 0
                    for kc in range(q1 + 1):
                        kidx = (kc % 2) if b == 0 else (1 - kc % 2)
                        for sub in range(2):
                            c0 = kidx * CH + sub * P
                            ps = att_s.tile([P, 2 * CH], f32, name="s_ps", tag="s")
                            nc.tensor.matmul(
                                ps[:], kp[kc // 2][:, c0 : c0 + P], qt[:],
                                start=True, stop=True,
                            )
                            E = qe_p.tile([P, 2 * CH], bf16, name="E", tag="E")
                            if kc == q1:
                                # future keys for q0-half: zero; diagonal for q1-half
                                nc.scalar.activation(
                                    E[:, CH : 2 * CH], ps[:, CH : 2 * CH],
                                    AFT.Exp, scale=0.125,
                                )
                                nc.vector.memset(E[:, 0:CH], 0.0)
                                nc.vector.tensor_tensor(
                                    E[:, CH : 2 * CH], E[:, CH : 2 * CH],
                                    c["tri"][sub][:], Alu.mult,
                                )
                            else:
                                nc.scalar.activation(E[:], ps[:], AFT.Exp, scale=0.125)
                                if kc == q0:  # diagonal for q0-half only
                                    nc.vector.tensor_tensor(
                                        E[:, 0:CH], E[:, 0:CH], c["tri"][sub][:], Alu.mult
                                    )
                            nc.tensor.matmul(
                                po[:],
                                v_ts[kc * 2 + sub][:, lh * 65 : (lh + 1) * 65],
                                E[:],
                                start=(mi == 0), stop=(mi == n_mm - 1),
                            )
                            mi += 1
                    # stage unnormalized O + denominator row (row 64)
                    for half, sq in ((0, s0), (1, s1)):
                        nc.vector.tensor_copy(
                            stage[sq][:, lh * CH : (lh + 1) * CH],
                            po[:, half * CH : (half + 1) * CH],
                        )
            for j in range(8):
                dst = bass.AP(
                    a2x, j * SLOT2 * CH, [[CH, 65], [65 * CH, 2], [1, CH]]
                )
                nc.sync.dma_start(dst, stage[j][:])
            nc.gpsimd.collective_compute(
                "AllToAll", mybir.AluOpType.bypass, replica_groups=rg,
                ins=[a2x.ap().opt()], outs=[a2o[b].ap().opt()],
            )
    att_scope.close()  # frees att psum banks + k/v/q sbuf pools

    # ================= post-norm + proj + residual1 =================
    from concourse.bass import _add_dep_helper

    def gate(dma_inst, anchor):
        """Order a weight-prefetch DMA after the attention input loads so it
        doesn't hog the DMA queues during the collective window."""
        _add_dep_helper(
            dma_inst.ins, anchor.ins, sync=False, reason="defer weight prefetch"
        )

    pools["x1"] = top.enter_context(tc.tile_pool(name="x1", bufs=8))
    x1_tiles = []
    with tc.tile_pool(name="wp", bufs=16) as wp_p, tc.tile_pool(
        name="onrm", bufs=6
    ) as onrm_p, tc.tile_pool(name="obf", bufs=8) as obf_p, tc.tile_pool(
        name="nrm_ps", bufs=2, space="PSUM"
    ) as nrm_ps:
        wpt = {}
        for dk in range(8):
            for db in range(2):
                wt = wp_p.tile([P, 512], bf16, name="wp", tag="wp")
                gate(
                    nc.sync.dma_start(
                        wt[:], wpT[dk * P : (dk + 1) * P, db * 512 : (db + 1) * 512]
                    ),
                    load_anchor[0],
                )
                wpt[(dk, db)] = wt
        o_bf = []
        for dk in range(8):
            t = obf_p.tile([P, T], bf16, name="obf", tag="obf")
            o_bf.append(t)
        for hb in range(2):
            a2x = a2o[hb]
            # denominators: [16, CH] (core c' rows lh*65+64)
            den_bf = onrm_p.tile([16, CH], bf16, name="den_bf", tag="den_bf")
            dsrc = bass.AP(a2x, DH * CH, [[SLOT2 * CH, 8], [65 * CH, 2], [1, CH]])
            nc.sync.dma_start(den_bf[:], dsrc)
            den = onrm_p.tile([16, CH], f32, name="den", tag="den")
            nc.vector.tensor_copy(den[:], den_bf[:])
            rec = onrm_p.tile([16, CH], f32, name="rec", tag="den")
            nc.vector.reciprocal(rec[:], den[:])
            rec_c = onrm_p.tile([16, CH], bf16, name="rec_c", tag="den_bf")
            with nc.allow_low_precision(reason="softmax denom bcast"):
                nc.vector.tensor_copy(rec_c[:], rec[:])
            for dk in range(8):
                o_un = onrm_p.tile([P, CH], bf16, name="o_un", tag="o_un")
                osrc = bass.AP(a2x, dk * SLOT2 * CH, [[65 * CH, 2], [CH, DH], [1, CH]])
                nc.sync.dma_start(o_un[:], osrc)
                rec_b = nrm_ps.tile([P, CH], f32, name="rec_b", tag="rec")
                nc.tensor.matmul(
                    rec_b[:], c["sel"][:, dk * P : (dk + 1) * P], rec_c[:],
                    start=True, stop=True,
                )
                nc.vector.tensor_tensor(
                    o_bf[dk][:, hb * CH : (hb + 1) * CH], o_un[:], rec_b[:], Alu.mult
                )
        x1_tiles = [
            pools["x1"].tile([P, T], f32, name="x1", tag="x1") for _ in range(8)
        ]
        for hb in range(2):  # proj per half so half A overlaps A2A#2B
            cols = slice(hb * CH, (hb + 1) * CH)
            for do in range(8):
                ps = mm_ps.tile([P, CH], f32, name="p_ps", tag="mm")
                for dk in range(8):
                    nc.tensor.matmul(
                        ps[:],
                        wpt[(dk, do // 4)][:, (do % 4) * P : (do % 4 + 1) * P],
                        o_bf[dk][:, cols],
                        start=(dk == 0), stop=(dk == 7),
                    )
                x1 = x1_tiles[do]
                nc.vector.tensor_scalar(
                    x1[:, cols], ps[:], c["bp"][:, do : do + 1], None, Alu.add
                )
                nc.vector.tensor_tensor(
                    x1[:, cols], x1[:, cols], x_tiles[do][:, cols], Alu.add
                )
    # ================= LN2 =================
    pools["xbb"] = top.enter_context(tc.tile_pool(name="xbb", bufs=8))
    pools["h2"] = top.enter_context(tc.tile_pool(name="h2", bufs=8))
    h2_tiles = layer_norm_T(x1_tiles, None, pools["h2"], "b")

    # ================= MLP up + gelu =================
    pools["gu"] = top.enter_context(tc.tile_pool(name="gu", bufs=32))
    gu_tiles = []
    with tc.tile_pool(name="wu", bufs=16) as wu_p:
        for jb in range(8):
            wut = []
            for dk in range(8):
                wt = wu_p.tile([P, 512], bf16, name="wu", tag="wu")
                gate(
                    nc.sync.dma_start(
                        wt[:], wuT[dk * P : (dk + 1) * P, jb * 512 : (jb + 1) * 512]
                    ),
                    load_anchor[1],
                )
                wut.append(wt)
            for jl in range(4):
                j = jb * 4 + jl
                ps = mm_ps.tile([P, T], f32, name="u_ps", tag="mm")
                for dk in range(8):
                    nc.tensor.matmul(
                        ps[:], wut[dk][:, jl * P : (jl + 1) * P], h2_tiles[dk][:],
                        start=(dk == 0), stop=(dk == 7),
                    )
                gu = pools["gu"].tile([P, T], bf16, name="gu", tag="gu")
                nc.scalar.activation(
                    gu[:], ps[:], AFT.Gelu_apprx_tanh, bias=c["bu"][:, j : j + 1]
                )
                gu_tiles.append(gu)

    # ================= MLP down + residual2 =================
    pools["outp"] = top.enter_context(tc.tile_pool(name="outp", bufs=4))
    with tc.tile_pool(name="wd", bufs=34) as wd_p:
        for db in range(2):
            wdt = []
            for j in range(32):
                wt = wd_p.tile([P, 512], bf16, name="wd", tag="wd")
                gate(
                    nc.sync.dma_start(
                        wt[:], wdT[j * P : (j + 1) * P, db * 512 : (db + 1) * 512]
                    ),
                    load_anchor[1],
                )
                wdt.append(wt)
            for dol in range(4):
                do = db * 4 + dol
                ps = mm_ps.tile([P, T], f32, name="d_ps", tag="mm")
                for j in range(32):
                    nc.tensor.matmul(
                        ps[:], wdt[j][:, dol * P : (dol + 1) * P], gu_tiles[j][:],
                        start=(j == 0), stop=(j == 31),
                    )
                o = pools["outp"].tile([P, T], f32, name="out_t", tag="out_t")
                nc.vector.tensor_scalar(
                    o[:], ps[:], c["bd"][:, do : do + 1], None, Alu.add
                )
                nc.vector.tensor_tensor(o[:], o[:], x1_tiles[do][:], Alu.add)
                nc.sync.dma_start(out[do * P : (do + 1) * P, :], o[:])


def _build():
    from contextlib import ExitStack
    from concourse import bass, mybir, tile, bacc

    f32 = mybir.dt.float32
    bf16 = mybir.dt.bfloat16

    nc = bacc.Bacc("TRN2", target_bir_lowering=False, num_devices=NCORES)

    xT = nc.declare_dram_parameter("xT", [D, T], f32, isOutput=False)
    wT = nc.declare_dram_parameter("wT", [D, 3 * D], bf16, isOutput=False)
    wpT = nc.declare_dram_parameter("wpT", [D, D], bf16, isOutput=False)
    wuT = nc.declare_dram_parameter("wuT", [D, DFF], bf16, isOutput=False)
    wdT = nc.declare_dram_parameter("wdT", [DFF, D], bf16, isOutput=False)
    bqk = nc.declare_dram_parameter("bqk", [P, 16], f32, isOutput=False)
    bv = nc.declare_dram_parameter("bv", [P, D], f32, isOutput=False)
    bp = nc.declare_dram_parameter("bp", [P, 8], f32, isOutput=False)
    bu = nc.declare_dram_parameter("bu", [P, 32], f32, isOutput=False)
    bd = nc.declare_dram_parameter("bd", [P, 8], f32, isOutput=False)
    tri = nc.declare_dram_parameter("tri", [CH, CH], bf16, isOutput=False)
    sel = nc.declare_dram_parameter("sel", [16, D], bf16, isOutput=False)
    out = nc.declare_dram_parameter("out", [D, T], f32, isOutput=True)

    a1i = {hb: nc.dram_tensor(f"a2a1_in{hb}", [NCORES * SLOT, CH], bf16) for hb in range(2)}
    a1o = {hb: nc.dram_tensor(f"a2a1_out{hb}", [NCORES * SLOT, CH], bf16) for hb in range(2)}
    a2i = {hb: nc.dram_tensor(f"a2a2_in{hb}", [NCORES * SLOT2, CH], bf16) for hb in range(2)}
    a2o = {hb: nc.dram_tensor(f"a2a2_out{hb}", [NCORES * SLOT2, CH], bf16) for hb in range(2)}

    with tile.TileContext(nc) as tc, ExitStack() as top:
        const = top.enter_context(tc.tile_pool(name="const", bufs=1))
        ones_d = const.tile([P, 1], bf16)
        nc.vector.memset(ones_d[:], 1.0 / D)
        ones_row = const.tile([1, P], bf16)
        nc.vector.memset(ones_row[:], 1.0)
        tri_t = [const.tile([P, CH], bf16, name=f"tri{s}", tag=f"tri{s}") for s in range(2)]
        for s in range(2):
            nc.sync.dma_start(tri_t[s][:], tri[s * P : (s + 1) * P, :])
        sel_t = const.tile([16, D], bf16, name="sel_t", tag="sel_t")
        nc.sync.dma_start(sel_t[:], sel[:, :])

        def ctile(name, param, shape):
            t = const.tile(shape, f32, name=name, tag=name)
            nc.sync.dma_start(t[:], param[:, :])
            return t

        consts = {
            "ones_d": ones_d, "ones_row": ones_row, "tri": tri_t, "sel": sel_t,
            "bqk": ctile("bqk_t", bqk, [P, 16]),
            "bv": ctile("bv_t", bv, [P, D]),
            "bp": ctile("bp_t", bp, [P, 8]),
            "bu": ctile("bu_t", bu, [P, 32]),
            "bd": ctile("bd_t", bd, [P, 8]),
        }

        pools = {
            "vec": top.enter_context(tc.tile_pool(name="vec", bufs=4)),
            "xt": top.enter_context(tc.tile_pool(name="xt", bufs=8)),
            "xba": top.enter_context(tc.tile_pool(name="xba", bufs=8)),
            "ht": top.enter_context(tc.tile_pool(name="ht", bufs=8)),
        }

        env = {
            "params": (xT, wT, wpT, wuT, wdT, out),
            "bounce": (a1i, a1o, a2i, a2o),
            "consts": consts,
            "pools": pools,
            "top": top,
        }
        _emit(nc, tc, env)

    nc.finalize()
    return nc


def _get_nc():
    if "nc" not in _CACHE:
        _CACHE["nc"] = _build()
    return _CACHE["nc"]


def _make_in_maps(inputs):
    x = np.asarray(inputs["x"], np.float32)
    ln1_g = np.asarray(inputs["ln1_g"], np.float32)
    ln1_b = np.asarray(inputs["ln1_b"], np.float32)
    W_attn = np.asarray(inputs["W_attn"], np.float32)
    b_attn = np.asarray(inputs["b_attn"], np.float32)
    W_proj = np.asarray(inputs["W_proj"], np.float32)
    b_proj = np.asarray(inputs["b_proj"], np.float32)
    ln2_g = np.asarray(inputs["ln2_g"], np.float32)
    ln2_b = np.asarray(inputs["ln2_b"], np.float32)
    W_up = np.asarray(inputs["W_up"], np.float32)
    b_up = np.asarray(inputs["b_up"], np.float32)
    W_down = np.asarray(inputs["W_down"], np.float32)
    b_down = np.asarray(inputs["b_down"], np.float32)

    bf = ml_dtypes.bfloat16

    # fold LN gamma/beta into the consuming weights/biases
    Wa = W_attn * ln1_g[None, :]
    ba = b_attn + W_attn @ ln1_b
    Wu = W_up * ln2_g[None, :]
    bu_ = b_up + W_up @ ln2_b

    wT = np.ascontiguousarray(Wa.T).astype(bf)
    wpT = np.ascontiguousarray(W_proj.T).astype(bf)
    wuT = np.ascontiguousarray(Wu.T).astype(bf)
    wdT = np.ascontiguousarray(W_down.T).astype(bf)

    def cols(v):  # [N] -> [128, N//128]: col j = v[j*128:(j+1)*128]
        return np.ascontiguousarray(v.reshape(-1, P).T).astype(np.float32)

    tri = np.tril(np.ones((CH, CH), np.float32)).T.astype(bf)  # tri[a,b] = a<=b
    tri = np.ascontiguousarray(tri)

    # sel[h, d] = 1 if head h owns output dim d (d//64 == h)
    sel = np.zeros((16, D), np.float32)
    for h in range(16):
        sel[h, h * DH : (h + 1) * DH] = 1.0
    sel = sel.astype(bf)

    common = dict(
        wT=wT, wpT=wpT, wuT=wuT, wdT=wdT, tri=tri, sel=sel,
        bqk=cols(ba[: 2 * D]),
        bv=np.ascontiguousarray(np.broadcast_to(ba[2 * D :].reshape(1, D), (P, D))).astype(np.float32),
        bp=cols(b_proj), bu=cols(bu_), bd=cols(b_down),
    )

    in_maps = []
    for i in range(NCORES):
        c0 = x[0, i * CH : (i + 1) * CH]  # [256, 1024]
        c1 = x[1, (7 - i) * CH : (8 - i) * CH]
        xTi = np.ascontiguousarray(np.concatenate([c0, c1], 0).T)  # [1024, 512]
        in_maps.append(dict(common, xT=xTi))
    return in_maps


def kernel(**inputs):
    in_maps = _make_in_maps(inputs)

    from concourse import bass_utils

    nc = _get_nc()
    res = bass_utils.run_bass_kernel_spmd(
        nc, in_maps, core_ids=list(range(NCORES)), trace=TRACE
    )
    _CACHE["last_res"] = res
    y = np.empty((B, S, D), np.float32)
    for i in range(NCORES):
        o = np.asarray(res.results[i]["out"], np.float32)  # [1024, 512]
        y[0, i * CH : (i + 1) * CH] = o[:, :CH].T
        y[1, (7 - i) * CH : (8 - i) * CH] = o[:, CH:].T
    return y
